# revision 21
# baseline (speedup 1.0000x reference)
"""Trainium2 Bass kernel for nn_DetectionConfidenceMap2keypoint.

Computes, for two [16,64,128,128] f32 heatmap tensors:
  map_val = |hm|                       (only for the first tensor)
  zeta[b,c]   = sum_{h,w} |hm|
  kpx[b,c]    = sum_{h,w} w * |hm|
  kpy[b,c]    = sum_{h,w} h * |hm|
  keypoint    = round([kpx/zeta, kpy/zeta])

Sharding: pure data parallel over batch B=16 across 8 NeuronCores
(2 batches/core -> 2*64 = 128 (b,c) pairs = the 128 SBUF partitions).

Device computes map_val and the three per-(b,c) sums; host does the
final divide + round (tiny [B,C] math, bit-identical np.round vs
jnp.round).
"""

import sys

if "/opt/trn_rl_repo" not in sys.path:
    sys.path.insert(0, "/opt/trn_rl_repo")

import numpy as np

import concourse.bacc as bacc
import concourse.mybir as mybir
import concourse.tile as tile
from concourse.bass_utils import run_bass_kernel_spmd

N_CORES = 8
B, C, H, W = 16, 64, 128, 128
BLOC = B // N_CORES            # batches per core = 2
P = BLOC * C                   # partitions used = 128
HW = H * W                     # 16384 elements per heatmap
NCH = 4                        # chunks per tensor block
F = HW // NCH                  # 2048 free elems per chunk
HCH = F // W                   # 16 heatmap rows per chunk

_CACHE = {}

# test.py reads this after a traced run
last_results = None


def _build_nc(
    repeat=1,
    skip_colsum=False,
    skip_rowsum=False,
    skip_mapstore=False,
    nch=NCH,
    store_engine="gpsimd",
    io_bufs=4,
):
    dt = mybir.dt.float32
    f = HW // nch
    hch = f // W
    nc = bacc.Bacc(None, target_bir_lowering=False, debug=False)

    hm = nc.dram_tensor("hm", [BLOC, C, H, W], dt, kind="ExternalInput")
    tfm = nc.dram_tensor("tfm", [BLOC, C, H, W], dt, kind="ExternalInput")
    map_out = nc.dram_tensor("map_out", [BLOC, C, H, W], dt, kind="ExternalOutput")
    sums_hm = nc.dram_tensor("sums_hm", [P, 3], dt, kind="ExternalOutput")
    sums_tf = nc.dram_tensor("sums_tf", [P, 3], dt, kind="ExternalOutput")

    # index weights: row-invariant [128, 128] tile, value j at free pos j
    idx = np.broadcast_to(np.arange(W, dtype=np.float32), (P, W)).copy()
    idx_dram = nc.inline_tensor(idx, name="idx_w")

    with tile.TileContext(nc) as tc:
        with (
            tc.tile_pool(name="io", bufs=io_bufs) as io_pool,
            tc.tile_pool(name="absb", bufs=io_bufs) as abs_pool,
            tc.tile_pool(name="acc", bufs=2) as acc_pool,
            tc.tile_pool(name="small", bufs=2) as small_pool,
            tc.tile_pool(name="singles", bufs=1) as singles,
        ):
            wf = singles.tile([P, W], dt)
            nc.sync.dma_start(out=wf, in_=idx_dram[:, :])

            for src, sums_dst, want_map in [
                (hm, sums_hm, True),
                (tfm, sums_tf, False),
            ] * repeat:
                x = src[:, :, :, :].rearrange("b c h w -> (b c) (h w)")
                mo = (
                    map_out[:, :, :, :].rearrange("b c h w -> (b c) (h w)")
                    if want_map
                    else None
                )
                # per-h row sums (accumulated by column range, no conflicts)
                R = acc_pool.tile([P, H], dt, tag="R")
                # per-chunk partial column sums
                Cp = acc_pool.tile([P, nch, W], dt, tag="Cp")

                for g in range(nch):
                    xin = io_pool.tile([P, f], dt, tag="xin")
                    nc.sync.dma_start(out=xin, in_=x[:, g * f : (g + 1) * f])
                    if want_map:
                        # abs on ScalarE; reduces read the abs output so the
                        # input tile has a single consumer (keeps HWDGE DMA
                        # sync-wait counts <= 1); the map store goes out via
                        # SWDGE so it doesn't contend with the load ring.
                        ab = abs_pool.tile([P, f], dt, tag="ab")
                        nc.scalar.activation(
                            out=ab, in_=xin, func=mybir.ActivationFunctionType.Abs
                        )
                        if not skip_mapstore:
                            st_eng = getattr(nc, store_engine)
                            st_eng.dma_start(
                                out=mo[:, g * f : (g + 1) * f], in_=ab
                            )
                        red_src = ab
                    else:
                        red_src = xin
                    if not skip_rowsum:
                        x3 = red_src[:, :].rearrange("p (h w) -> p h w", w=W)
                        nc.vector.tensor_reduce(
                            out=R[:, g * hch : (g + 1) * hch],
                            in_=x3,
                            axis=mybir.AxisListType.X,
                            op=mybir.AluOpType.add,
                            apply_absolute_value=True,
                        )
                    if not skip_colsum:
                        x3t = red_src[:, :].rearrange("p (h w) -> p w h", w=W)
                        nc.vector.tensor_reduce(
                            out=Cp[:, g, :],
                            in_=x3t,
                            axis=mybir.AxisListType.X,
                            op=mybir.AluOpType.add,
                            apply_absolute_value=True,
                        )

                S = small_pool.tile([P, 3], dt, tag="S")
                if skip_rowsum or skip_colsum:
                    nc.vector.memset(S, 0.0)
                if not skip_colsum:
                    Ct = small_pool.tile([P, W], dt, tag="Ct")
                    scr0 = small_pool.tile([P, W], dt, tag="scr0")
                    # total column sums from per-chunk partials
                    nc.vector.tensor_reduce(
                        out=Ct,
                        in_=Cp[:, :, :].rearrange("p g w -> p w g"),
                        axis=mybir.AxisListType.X,
                        op=mybir.AluOpType.add,
                    )
                    # kpx = sum_w w * colsum[w]
                    # (tensor_tensor_reduce is a custom DVE op that crashes
                    # the exec unit under the PJRT path - use mul + reduce)
                    nc.vector.tensor_mul(out=scr0, in0=Ct, in1=wf)
                    nc.vector.tensor_reduce(
                        out=S[:, 1:2],
                        in_=scr0,
                        axis=mybir.AxisListType.X,
                        op=mybir.AluOpType.add,
                    )
                if not skip_rowsum:
                    scr1 = small_pool.tile([P, H], dt, tag="scr1")
                    # zeta
                    nc.vector.tensor_reduce(
                        out=S[:, 0:1],
                        in_=R,
                        axis=mybir.AxisListType.X,
                        op=mybir.AluOpType.add,
                    )
                    # kpy = sum_h h * rowsum[h]
                    nc.vector.tensor_mul(out=scr1, in0=R, in1=wf)
                    nc.vector.tensor_reduce(
                        out=S[:, 2:3],
                        in_=scr1,
                        axis=mybir.AxisListType.X,
                        op=mybir.AluOpType.add,
                    )
                nc.gpsimd.dma_start(out=sums_dst[:, :], in_=S)

    nc.finalize()
    return nc


def _get_nc():
    if "nc" not in _CACHE:
        _CACHE["nc"] = _build_nc()
    return _CACHE["nc"]


def _finish(sums):
    """sums: [P, 3] device output -> (zeta [BLOC,C], keypoint [BLOC,C,2])."""
    zeta = sums[:, 0].reshape(BLOC, C)
    kx = sums[:, 1].reshape(BLOC, C)
    ky = sums[:, 2].reshape(BLOC, C)
    kp = np.round(np.stack([kx / zeta, ky / zeta], axis=-1)).astype(np.float32)
    return zeta, kp


def kernel(combined_hm_preds, tf_combined_hm_preds, cur_batch=None, **_kw):
    global last_results
    hm = np.ascontiguousarray(np.asarray(combined_hm_preds, dtype=np.float32))
    tfm = np.ascontiguousarray(np.asarray(tf_combined_hm_preds, dtype=np.float32))
    assert hm.shape == (B, C, H, W), hm.shape

    nc = _get_nc()
    in_maps = [
        {
            "hm": np.ascontiguousarray(hm[i * BLOC : (i + 1) * BLOC]),
            "tfm": np.ascontiguousarray(tfm[i * BLOC : (i + 1) * BLOC]),
        }
        for i in range(N_CORES)
    ]
    res = run_bass_kernel_spmd(nc, in_maps, core_ids=list(range(N_CORES)))
    last_results = res

    maps, zs, kps, tkps = [], [], [], []
    for r in res.results:
        maps.append(r["map_out"])
        z, k = _finish(r["sums_hm"])
        _, tk = _finish(r["sums_tf"])
        zs.append(z)
        kps.append(k)
        tkps.append(tk)

    map_val_all = np.concatenate(maps, axis=0)
    keypoint = np.concatenate(kps, axis=0)
    get_zeta = np.concatenate(zs, axis=0)
    tf_keypoint = np.concatenate(tkps, axis=0)
    return map_val_all, keypoint, get_zeta, tf_keypoint
